# revision 1
# baseline (speedup 1.0000x reference)
"""Trainium2 Bass kernel for nn_DiagonalLayer (per-gene weighted feature sum).

out[b, g] = sum_f x[b, 3g+f] * w[3g+f] + bias[g]

Sharding: data-parallel over the batch dim — 4096 rows split as 512 rows on
each of the 8 NeuronCores; w/bias replicated (tiny). Output gathered by
concatenation along batch.

Self-contained: hardcodes shapes; only imports the concourse toolchain.
"""

import sys

import numpy as np

if "/opt/trn_rl_repo" not in sys.path:
    sys.path.insert(0, "/opt/trn_rl_repo")

B = 4096
GF = 27687
G = 9229
F = 3
NCORES = 8
BSH = B // NCORES  # 512 batch rows per core
PT = 128  # SBUF partitions
NT = BSH // PT  # 4 batch tiles per core
GC = 2308  # genes per chunk (v1)

# v2 knobs
V2_GC = 2048  # genes per chunk
V2_SPLIT = 0.68  # fraction of genes computed on DVE (rest on GpSimd)
V2_DVE_METHOD = "reduce"  # "adds" (strided) or "reduce"

import os as _os

VARIANT = _os.environ.get("KERNEL_VARIANT", "v2")

_cached_nc = None


def _gene_chunks(gc_size=GC):
    chunks = []
    c0 = 0
    while c0 < G:
        gc = min(gc_size, G - c0)
        chunks.append((c0, gc))
        c0 += gc
    return chunks


def _build_nc():
    import concourse.bacc as bacc
    import concourse.mybir as mybir
    import concourse.tile as tile

    f32 = mybir.dt.float32
    nc = bacc.Bacc(
        "TRN2", target_bir_lowering=False, debug=False, num_devices=NCORES
    )
    x = nc.dram_tensor("x", [BSH, GF], f32, kind="ExternalInput")
    w = nc.dram_tensor("w", [GF], f32, kind="ExternalInput")
    bias = nc.dram_tensor("bias", [G], f32, kind="ExternalInput")
    out = nc.dram_tensor("out", [BSH, G], f32, kind="ExternalOutput")

    if VARIANT == "v1":
        _emit_v1(nc, tile, mybir, f32, x, w, bias, out)
    else:
        _emit_v2(nc, tile, mybir, f32, x, w, bias, out)
    if not nc.is_finalized():
        nc.finalize()
    return nc


def _emit_v2(nc, tile, mybir, f32, x, w, bias, out):
    """Gene-split variant: per (chunk, batch-tile) iteration, DVE computes the
    first V2_SPLIT fraction of genes (mult + strided segment adds + bias) while
    GpSimd computes the rest. w/bias are broadcast across partitions via
    TensorE ones-matmul + ScalarE PSUM->SBUF copy, keeping DMA rings free."""
    with tile.TileContext(nc) as tc:
        with (
            tc.tile_pool(name="const", bufs=1) as const_pool,
            tc.tile_pool(name="wrow", bufs=2) as row_pool,
            tc.tile_pool(name="psum", bufs=6, space="PSUM") as psum_pool,
            tc.tile_pool(name="wb", bufs=2) as wb_pool,
            tc.tile_pool(name="bb", bufs=2) as bb_pool,
            tc.tile_pool(name="xa", bufs=3) as xa_pool,
            tc.tile_pool(name="xb", bufs=3) as xb_pool,
            tc.tile_pool(name="oa", bufs=4) as oa_pool,
            tc.tile_pool(name="ob", bufs=4) as ob_pool,
        ):
            ones = const_pool.tile([1, PT], f32, tag="ones")
            nc.vector.memset(ones[:, :], 1.0)

            ROW = 1024  # row-load granularity (two 512-wide matmuls per row)

            def bcast(dst, src_dram, off, n_total):
                # dst[p, j] = src_dram[off + j] for all 128 partitions
                for o in range(0, n_total, ROW):
                    n = min(ROW, n_total - o)
                    row = row_pool.tile([1, ROW], f32, tag="wrow")
                    nc.sync.dma_start(
                        out=row[:1, :n], in_=src_dram[None, off + o : off + o + n]
                    )
                    for o2 in range(0, n, 512):
                        n2 = min(512, n - o2)
                        ps = psum_pool.tile([PT, 512], f32, tag="ps")
                        nc.tensor.matmul(
                            ps[:, :n2], ones[:1, :], row[:1, o2 : o2 + n2]
                        )
                        nc.scalar.copy(dst[:, o + o2 : o + o2 + n2], ps[:, :n2])

            chunks = _gene_chunks(V2_GC)

            def bcast_chunk(c0, gc):
                wbt = wb_pool.tile([PT, F * gc], f32, tag="wb")
                bcast(wbt, w, F * c0, F * gc)
                bbt = bb_pool.tile([PT, gc], f32, tag="bb")
                bcast(bbt, bias, c0, gc)
                return wbt, bbt

            cur = bcast_chunk(*chunks[0])
            for ci, (c0, gc) in enumerate(chunks):
                wbt, bbt = cur
                s = int(round(gc * V2_SPLIT))
                nb = gc - s

                for t in range(NT):
                    rows = slice(t * PT, (t + 1) * PT)
                    # --- DVE range: genes [c0, c0+s) ---
                    xa_t = xa_pool.tile([PT, F * s], f32, tag="xa")
                    nc.sync.dma_start(
                        out=xa_t[:, :], in_=x[rows, F * c0 : F * (c0 + s)]
                    )
                    oa_t = oa_pool.tile([PT, s], f32, tag="oa")
                    nc.vector.tensor_mul(xa_t[:, :], xa_t[:, :], wbt[:, : F * s])
                    y3 = xa_t[:, :].rearrange("p (g f) -> p g f", f=F)
                    if V2_DVE_METHOD == "adds":
                        nc.vector.tensor_add(oa_t[:, :], y3[:, :, 0], y3[:, :, 1])
                        nc.vector.tensor_add(oa_t[:, :], oa_t[:, :], y3[:, :, 2])
                    else:
                        nc.vector.reduce_sum(
                            oa_t[:, :], y3, axis=mybir.AxisListType.X
                        )
                    nc.vector.tensor_add(oa_t[:, :], oa_t[:, :], bbt[:, :s])
                    # stores go on the ACT HWDGE queue so the SP queue (x
                    # loads) never blocks behind a compute-dependent store
                    nc.scalar.dma_start(out=out[rows, c0 : c0 + s], in_=oa_t[:, :])

                    # --- GpSimd range: genes [c0+s, c0+gc) ---
                    xb_t = xb_pool.tile([PT, F * nb], f32, tag="xb")
                    nc.sync.dma_start(
                        out=xb_t[:, :], in_=x[rows, F * (c0 + s) : F * (c0 + gc)]
                    )
                    ob_t = ob_pool.tile([PT, nb], f32, tag="ob")
                    nc.gpsimd.tensor_mul(
                        xb_t[:, :], xb_t[:, :], wbt[:, F * s : F * gc]
                    )
                    z3 = xb_t[:, :].rearrange("p (g f) -> p g f", f=F)
                    nc.gpsimd.tensor_add(ob_t[:, :], z3[:, :, 0], z3[:, :, 1])
                    nc.gpsimd.tensor_add(ob_t[:, :], ob_t[:, :], z3[:, :, 2])
                    nc.gpsimd.tensor_add(ob_t[:, :], ob_t[:, :], bbt[:, s:gc])
                    nc.scalar.dma_start(
                        out=out[rows, c0 + s : c0 + gc], in_=ob_t[:, :]
                    )

                    if t == 0 and ci + 1 < len(chunks):
                        # emit the next chunk's broadcast early so its row
                        # loads / matmuls / copies dispatch while this chunk
                        # is still computing
                        cur = bcast_chunk(*chunks[ci + 1])


def _emit_v1(nc, tile, mybir, f32, x, w, bias, out):
    with tile.TileContext(nc) as tc:
        with (
            tc.tile_pool(name="wb", bufs=2) as wb_pool,
            tc.tile_pool(name="bb", bufs=2) as bb_pool,
            tc.tile_pool(name="xc", bufs=3) as x_pool,
            tc.tile_pool(name="oc", bufs=3) as o_pool,
        ):
            for c0, gc in _gene_chunks():
                wbt = wb_pool.tile([PT, F * gc], f32, tag="wb")
                nc.sync.dma_start(
                    out=wbt[:1, :], in_=w[None, F * c0 : F * (c0 + gc)]
                )
                nc.gpsimd.partition_broadcast(wbt[:, :], wbt[:1, :])

                bbt = bb_pool.tile([PT, gc], f32, tag="bb")
                nc.sync.dma_start(out=bbt[:1, :], in_=bias[None, c0 : c0 + gc])
                nc.gpsimd.partition_broadcast(bbt[:, :], bbt[:1, :])

                for t in range(NT):
                    xc = x_pool.tile([PT, F * gc], f32, tag="xc")
                    nc.sync.dma_start(
                        out=xc[:, :],
                        in_=x[t * PT : (t + 1) * PT, F * c0 : F * (c0 + gc)],
                    )
                    nc.vector.tensor_mul(xc[:, :], xc[:, :], wbt[:, :])
                    oc = o_pool.tile([PT, gc], f32, tag="oc")
                    x3 = xc[:, :].rearrange("p (g f) -> p g f", f=F)
                    nc.vector.reduce_sum(oc[:, :], x3, axis=mybir.AxisListType.X)
                    nc.vector.tensor_add(oc[:, :], oc[:, :], bbt[:, :])
                    nc.sync.dma_start(
                        out=out[t * PT : (t + 1) * PT, c0 : c0 + gc], in_=oc[:, :]
                    )


def _get_nc():
    global _cached_nc
    if _cached_nc is None:
        _cached_nc = _build_nc()
    return _cached_nc


def run(x, weights, bias, trace=False, tmpdir=None):
    from concourse.bass_utils import run_bass_kernel_spmd

    x = np.ascontiguousarray(np.asarray(x, dtype=np.float32))
    weights = np.ascontiguousarray(np.asarray(weights, dtype=np.float32))
    bias_np = np.ascontiguousarray(np.asarray(bias, dtype=np.float32))

    nc = _get_nc()
    in_maps = [
        {
            "x": np.ascontiguousarray(x[c * BSH : (c + 1) * BSH]),
            "w": weights,
            "bias": bias_np,
        }
        for c in range(NCORES)
    ]
    try:
        res = run_bass_kernel_spmd(
            nc, in_maps, list(range(NCORES)), trace=trace, tmpdir=tmpdir
        )
    except Exception:
        # transient NRT device errors (e.g. NRT_EXEC_UNIT_UNRECOVERABLE after
        # a wedged run) usually clear on retry
        res = run_bass_kernel_spmd(
            nc, in_maps, list(range(NCORES)), trace=trace, tmpdir=tmpdir
        )
    outs = [res.results[c]["out"] for c in range(NCORES)]
    full = np.concatenate(outs, axis=0)
    return full, res


def kernel(x, weights, bias):
    full, _ = run(x, weights, bias, trace=False)
    return full



# revision 10
# speedup vs baseline: 1.1175x; 1.1175x over previous
"""Trainium2 Bass kernel for nn_DiagonalLayer (per-gene weighted feature sum).

out[b, g] = sum_f x[b, 3g+f] * w[3g+f] + bias[g]

Sharding: data-parallel over batch — 512 rows per core on 8 NeuronCores;
w/bias replicated. Host repacks x so each 512-gene sub-chunk is one
contiguous [128, 4*3*512] DMA slab (t-major within sub-chunk).

v3 design (per core):
 - PE broadcasts w/bias across partitions per 512-gene sub-chunk via
   float32r ones-matmuls into PSUM (4 banks/sub, double-buffered).
 - DVE sub-chunks read w/bias FROM PSUM (2nd TT operand on the PSUM port,
   not the shared SBUF port) -> no DVE/GpSimd shared-port contention:
   mul TT + 1-port reduce_sum + bias TT.
 - GpSimd sub-chunks get w/bias copied PSUM->SBUF by the otherwise-idle
   ScalarE, then mul + strided segment adds.
 - x loads on the SP HWDGE ring; stores + copies on the ACT ring.
"""

import os as _os
import sys

import numpy as np

if "/opt/trn_rl_repo" not in sys.path:
    sys.path.insert(0, "/opt/trn_rl_repo")

B = 4096
GF = 27687
G = 9229
F = 3
NCORES = 8
BSH = B // NCORES  # 512 batch rows per core
PT = 128  # SBUF partitions
NT = BSH // PT  # 4 batch tiles per core

SUB = 512  # genes per PSUM sub-chunk (w 3 banks + bias 1 bank)
GPAD = ((G + SUB - 1) // SUB) * SUB  # 9728: w/bias zero-padded so every broadcast matmul is FD=512
VARIANT = _os.environ.get("KERNEL_VARIANT", "v2")
V3_NG = int(_os.environ.get("V3_NG", "7"))  # GpSimd sub-chunks (of 18 full)
V3_BCAST = _os.environ.get("V3_BCAST", "f32r")

_cached_nc = None


def _subs():
    out = []
    g0 = 0
    while g0 < G:
        gw = min(SUB, G - g0)
        out.append((g0, gw))
        g0 += gw
    return out


def _gsub_set():
    """Evenly spread V3_NG GpSimd subs among the 18 full subs (runt on DVE)."""
    nfull = len(_subs()) - 1
    return {int((j + 0.5) * nfull / V3_NG) for j in range(V3_NG)}


def _build_nc():
    import concourse.bacc as bacc
    import concourse.mybir as mybir
    import concourse.tile as tile

    f32 = mybir.dt.float32
    nc = bacc.Bacc(
        "TRN2", target_bir_lowering=False, debug=False, num_devices=NCORES
    )
    if VARIANT == "v3":
        fw = mybir.dt.float32r if V3_BCAST == "f32r" else f32
        x_t = nc.dram_tensor("x_t", [PT, NT * GF], f32, kind="ExternalInput")
        w = nc.dram_tensor("w", [F * GPAD], fw, kind="ExternalInput")
        bias = nc.dram_tensor("bias", [GPAD], fw, kind="ExternalInput")
        ones_in = nc.dram_tensor("ones_in", [PT], fw, kind="ExternalInput")
        out_t = nc.dram_tensor("out_t", [PT, NT * G], f32, kind="ExternalOutput")
        _emit_v3(nc, tile, mybir, f32, x_t, w, bias, ones_in, out_t)
    else:
        x = nc.dram_tensor("x", [BSH, GF], f32, kind="ExternalInput")
        w = nc.dram_tensor("w", [GF], f32, kind="ExternalInput")
        bias = nc.dram_tensor("bias", [G], f32, kind="ExternalInput")
        out = nc.dram_tensor("out", [BSH, G], f32, kind="ExternalOutput")
        _emit_v2(nc, tile, mybir, f32, x, w, bias, out)
    if not nc.is_finalized():
        nc.finalize()
    return nc


def _emit_v3(nc, tile, mybir, f32, x_t, w, bias, ones_in, out_t):
    fw = mybir.dt.float32r if V3_BCAST == "f32r" else f32
    subs = _subs()
    gset = _gsub_set()

    with tile.TileContext(nc) as tc:
        with (
            tc.tile_pool(name="const", bufs=1) as const_pool,
            tc.tile_pool(name="wrow", bufs=3) as wrow_pool,
            tc.tile_pool(name="brow", bufs=3) as brow_pool,
            tc.tile_pool(name="psum", bufs=2, space="PSUM") as psum_pool,
            tc.tile_pool(name="wg", bufs=2) as wg_pool,
            tc.tile_pool(name="x", bufs=4) as x_pool,
            tc.tile_pool(name="xwd", bufs=2) as xwd_pool,
            tc.tile_pool(name="xwg", bufs=2) as xwg_pool,
            tc.tile_pool(name="o", bufs=3) as o_pool,
        ):
            ones = const_pool.tile([1, PT], fw, tag="ones")
            nc.sync.dma_start(out=ones[:1, :], in_=ones_in[None, :])

            for si, (g0, gw) in enumerate(subs):
                cw = F * gw
                is_g = si in gset

                # x slab: [128, NT*cw], t-major within the sub-chunk
                xt = x_pool.tile([PT, NT * cw], f32, tag="x")
                nc.sync.dma_start(
                    out=xt[:, :], in_=x_t[:, NT * F * g0 : NT * F * g0 + NT * cw]
                )

                # broadcast rows + PE ones-matmuls into PSUM
                wr = wrow_pool.tile([1, F * SUB], fw, tag="wr")
                nc.sync.dma_start(
                    out=wr[:1, :], in_=w[None, F * SUB * si : F * SUB * (si + 1)]
                )
                br = brow_pool.tile([1, SUB], fw, tag="br")
                nc.sync.dma_start(
                    out=br[:1, :], in_=bias[None, SUB * si : SUB * (si + 1)]
                )
                ps = psum_pool.tile([PT, 4 * SUB], f32, tag="ps")
                for j in range(0, F * SUB, SUB):
                    nc.tensor.matmul(
                        ps[:, j : j + SUB], ones[:1, :], wr[:1, j : j + SUB]
                    )
                nc.tensor.matmul(ps[:, 3 * SUB : 4 * SUB], ones[:1, :], br[:1, :])

                if is_g:
                    # ScalarE copies w+bias PSUM->SBUF for GpSimd
                    wg = wg_pool.tile([PT, 4 * SUB], f32, tag="wg")
                    if cw == 3 * SUB:
                        nc.scalar.copy(wg[:, : 3 * SUB + gw], ps[:, : 3 * SUB + gw])
                    else:
                        nc.scalar.copy(wg[:, :cw], ps[:, :cw])
                        nc.scalar.copy(
                            wg[:, 3 * SUB : 3 * SUB + gw],
                            ps[:, 3 * SUB : 3 * SUB + gw],
                        )

                ot = o_pool.tile([PT, NT * gw], f32, tag="o")
                for t in range(NT):
                    xs = xt[:, t * cw : (t + 1) * cw]
                    osl = ot[:, t * gw : (t + 1) * gw]
                    if is_g:
                        xw = xwg_pool.tile([PT, cw], f32, tag="xwg")
                        nc.gpsimd.tensor_mul(xw[:, :], xs, wg[:, :cw])
                        z = xw[:, :].rearrange("p (g f) -> p g f", f=F)
                        nc.gpsimd.tensor_add(osl, z[:, :, 0], z[:, :, 1])
                        nc.gpsimd.tensor_add(osl, osl, z[:, :, 2])
                        nc.gpsimd.tensor_add(
                            osl, osl, wg[:, 3 * SUB : 3 * SUB + gw]
                        )
                    else:
                        xw = xwd_pool.tile([PT, cw], f32, tag="xwd")
                        nc.vector.tensor_mul(xw[:, :], xs, ps[:, :cw])
                        y = xw[:, :].rearrange("p (g f) -> p g f", f=F)
                        nc.vector.reduce_sum(osl, y, axis=mybir.AxisListType.X)
                        nc.vector.tensor_add(
                            osl, osl, ps[:, 3 * SUB : 3 * SUB + gw]
                        )
                nc.scalar.dma_start(
                    out=out_t[:, NT * g0 : NT * g0 + NT * gw], in_=ot[:, :]
                )


def _emit_v2(nc, tile, mybir, f32, x, w, bias, out):
    """Previous-generation kernel (fallback). Gene-split DVE/GpSimd with
    fp32 ones-matmul broadcast via PSUM + ScalarE copies."""
    V2_GC = 2048
    V2_SPLIT = 0.68

    def chunks():
        out_ = []
        c0 = 0
        while c0 < G:
            gc = min(V2_GC, G - c0)
            out_.append((c0, gc))
            c0 += gc
        return out_

    with tile.TileContext(nc) as tc:
        with (
            tc.tile_pool(name="const", bufs=1) as const_pool,
            tc.tile_pool(name="wrow", bufs=2) as row_pool,
            tc.tile_pool(name="psum", bufs=6, space="PSUM") as psum_pool,
            tc.tile_pool(name="wb", bufs=2) as wb_pool,
            tc.tile_pool(name="bb", bufs=2) as bb_pool,
            tc.tile_pool(name="xa", bufs=3) as xa_pool,
            tc.tile_pool(name="xb", bufs=3) as xb_pool,
            tc.tile_pool(name="oa", bufs=4) as oa_pool,
            tc.tile_pool(name="ob", bufs=4) as ob_pool,
        ):
            ones = const_pool.tile([1, PT], f32, tag="ones")
            nc.vector.memset(ones[:, :], 1.0)
            ROW = 1024

            def bcast(dst, src_dram, off, n_total):
                for o in range(0, n_total, ROW):
                    n = min(ROW, n_total - o)
                    row = row_pool.tile([1, ROW], f32, tag="wrow")
                    nc.sync.dma_start(
                        out=row[:1, :n], in_=src_dram[None, off + o : off + o + n]
                    )
                    for o2 in range(0, n, 512):
                        n2 = min(512, n - o2)
                        ps = psum_pool.tile([PT, 512], f32, tag="ps")
                        nc.tensor.matmul(
                            ps[:, :n2], ones[:1, :], row[:1, o2 : o2 + n2]
                        )
                        nc.scalar.copy(dst[:, o + o2 : o + o2 + n2], ps[:, :n2])

            def bcast_chunk(c0, gc):
                wbt = wb_pool.tile([PT, F * gc], f32, tag="wb")
                bcast(wbt, w, F * c0, F * gc)
                bbt = bb_pool.tile([PT, gc], f32, tag="bb")
                bcast(bbt, bias, c0, gc)
                return wbt, bbt

            chs = chunks()
            cur = bcast_chunk(*chs[0])
            for ci, (c0, gc) in enumerate(chs):
                wbt, bbt = cur
                s = int(round(gc * V2_SPLIT))
                nb = gc - s
                for t in range(NT):
                    rows = slice(t * PT, (t + 1) * PT)
                    xa_t = xa_pool.tile([PT, F * s], f32, tag="xa")
                    nc.sync.dma_start(
                        out=xa_t[:, :], in_=x[rows, F * c0 : F * (c0 + s)]
                    )
                    oa_t = oa_pool.tile([PT, s], f32, tag="oa")
                    nc.vector.tensor_mul(xa_t[:, :], xa_t[:, :], wbt[:, : F * s])
                    y3 = xa_t[:, :].rearrange("p (g f) -> p g f", f=F)
                    nc.vector.reduce_sum(oa_t[:, :], y3, axis=mybir.AxisListType.X)
                    nc.vector.tensor_add(oa_t[:, :], oa_t[:, :], bbt[:, :s])
                    nc.scalar.dma_start(out=out[rows, c0 : c0 + s], in_=oa_t[:, :])

                    xb_t = xb_pool.tile([PT, F * nb], f32, tag="xb")
                    nc.sync.dma_start(
                        out=xb_t[:, :], in_=x[rows, F * (c0 + s) : F * (c0 + gc)]
                    )
                    ob_t = ob_pool.tile([PT, nb], f32, tag="ob")
                    nc.gpsimd.tensor_mul(
                        xb_t[:, :], xb_t[:, :], wbt[:, F * s : F * gc]
                    )
                    z3 = xb_t[:, :].rearrange("p (g f) -> p g f", f=F)
                    nc.gpsimd.tensor_add(ob_t[:, :], z3[:, :, 0], z3[:, :, 1])
                    nc.gpsimd.tensor_add(ob_t[:, :], ob_t[:, :], z3[:, :, 2])
                    nc.gpsimd.tensor_add(ob_t[:, :], ob_t[:, :], bbt[:, s:gc])
                    nc.scalar.dma_start(
                        out=out[rows, c0 + s : c0 + gc], in_=ob_t[:, :]
                    )
                    if t == 0 and ci + 1 < len(chs):
                        cur = bcast_chunk(*chs[ci + 1])


def _get_nc():
    global _cached_nc
    if _cached_nc is None:
        _cached_nc = _build_nc()
    return _cached_nc


def _pack_x(xc):
    """[512, GF] -> [128, NT*GF]: per sub-chunk, t-major slabs."""
    x3 = xc.reshape(NT, PT, GF)
    slabs = []
    for g0, gw in _subs():
        blk = x3[:, :, F * g0 : F * (g0 + gw)]  # [NT, PT, cw]
        slabs.append(blk.transpose(1, 0, 2).reshape(PT, NT * F * gw))
    return np.ascontiguousarray(np.concatenate(slabs, axis=1))


def _unpack_out(ot):
    """[128, NT*G] -> [512, G]"""
    cols = []
    for g0, gw in _subs():
        blk = ot[:, NT * g0 : NT * (g0 + gw)].reshape(PT, NT, gw)
        cols.append(blk.transpose(1, 0, 2).reshape(BSH, gw))
    return np.concatenate(cols, axis=1)


def run(x, weights, bias, trace=False, tmpdir=None):
    from concourse.bass_utils import run_bass_kernel_spmd

    x = np.ascontiguousarray(np.asarray(x, dtype=np.float32))
    weights = np.ascontiguousarray(np.asarray(weights, dtype=np.float32))
    bias_np = np.ascontiguousarray(np.asarray(bias, dtype=np.float32))

    nc = _get_nc()
    if VARIANT == "v3":
        ones128 = np.ones(PT, dtype=np.float32)
        w_pad = np.zeros(F * GPAD, dtype=np.float32)
        w_pad[:GF] = weights
        b_pad = np.zeros(GPAD, dtype=np.float32)
        b_pad[:G] = bias_np
        in_maps = [
            {
                "x_t": _pack_x(x[c * BSH : (c + 1) * BSH]),
                "w": w_pad,
                "bias": b_pad,
                "ones_in": ones128,
            }
            for c in range(NCORES)
        ]
    else:
        in_maps = [
            {
                "x": np.ascontiguousarray(x[c * BSH : (c + 1) * BSH]),
                "w": weights,
                "bias": bias_np,
            }
            for c in range(NCORES)
        ]
    try:
        res = run_bass_kernel_spmd(
            nc, in_maps, list(range(NCORES)), trace=trace, tmpdir=tmpdir
        )
    except Exception:
        # transient NRT device errors usually clear on retry
        res = run_bass_kernel_spmd(
            nc, in_maps, list(range(NCORES)), trace=trace, tmpdir=tmpdir
        )
    if VARIANT == "v3":
        outs = [_unpack_out(res.results[c]["out_t"]) for c in range(NCORES)]
    else:
        outs = [res.results[c]["out"] for c in range(NCORES)]
    full = np.concatenate(outs, axis=0)
    return full, res


def kernel(x, weights, bias):
    full, _ = run(x, weights, bias, trace=False)
    return full


# revision 13
# speedup vs baseline: 1.1970x; 1.0712x over previous
"""Trainium2 Bass kernel for nn_DiagonalLayer (per-gene weighted feature sum).

out[b, g] = sum_f x[b, 3g+f] * w[3g+f] + bias[g]

Sharding: data-parallel over batch — 512 rows per core on 8 NeuronCores;
w/bias replicated. Host repacks x so each 512-gene sub-chunk is one
contiguous [128, 4*3*512] DMA slab (t-major within sub-chunk).

v3 design (per core):
 - PE broadcasts w/bias across partitions per 512-gene sub-chunk via
   float32r ones-matmuls into PSUM (4 banks/sub, double-buffered).
 - DVE sub-chunks read w/bias FROM PSUM (2nd TT operand on the PSUM port,
   not the shared SBUF port) -> no DVE/GpSimd shared-port contention:
   mul TT + 1-port reduce_sum + bias TT.
 - GpSimd sub-chunks get w/bias copied PSUM->SBUF by the otherwise-idle
   ScalarE, then mul + strided segment adds.
 - x loads on the SP HWDGE ring; stores + copies on the ACT ring.
"""

import os as _os
import sys

import numpy as np

if "/opt/trn_rl_repo" not in sys.path:
    sys.path.insert(0, "/opt/trn_rl_repo")

B = 4096
GF = 27687
G = 9229
F = 3
NCORES = 8
BSH = B // NCORES  # 512 batch rows per core
PT = 128  # SBUF partitions
NT = BSH // PT  # 4 batch tiles per core

SUB = 512  # genes per PSUM sub-chunk (w 3 banks + bias 1 bank)
GPAD = ((G + SUB - 1) // SUB) * SUB  # 9728: w/bias zero-padded so every broadcast matmul is FD=512
VARIANT = _os.environ.get("KERNEL_VARIANT", "v3")
V3_NG = int(_os.environ.get("V3_NG", "7"))  # GpSimd sub-chunks (of 18 full)
V3_BCAST = _os.environ.get("V3_BCAST", "f32r")
V3_MERGE = _os.environ.get("V3_MERGE", "0") == "1"  # one instr group per sub-chunk

_cached_nc = None


def _subs():
    out = []
    g0 = 0
    while g0 < G:
        gw = min(SUB, G - g0)
        out.append((g0, gw))
        g0 += gw
    return out


def _gsub_set():
    """Evenly spread V3_NG GpSimd subs among the 18 full subs (runt on DVE)."""
    nfull = len(_subs()) - 1
    return {int((j + 0.5) * nfull / V3_NG) for j in range(V3_NG)}


def _build_nc():
    import concourse.bacc as bacc
    import concourse.mybir as mybir
    import concourse.tile as tile

    f32 = mybir.dt.float32
    nc = bacc.Bacc(
        "TRN2", target_bir_lowering=False, debug=False, num_devices=NCORES
    )
    if VARIANT == "v3":
        fw = mybir.dt.float32r if V3_BCAST == "f32r" else f32
        x_t = nc.dram_tensor("x_t", [PT, NT * GF], f32, kind="ExternalInput")
        w = nc.dram_tensor("w", [F * GPAD], fw, kind="ExternalInput")
        bias = nc.dram_tensor("bias", [GPAD], fw, kind="ExternalInput")
        ones_in = nc.dram_tensor("ones_in", [PT], fw, kind="ExternalInput")
        out_t = nc.dram_tensor("out_t", [PT, NT * G], f32, kind="ExternalOutput")
        _emit_v3(nc, tile, mybir, f32, x_t, w, bias, ones_in, out_t)
    else:
        x = nc.dram_tensor("x", [BSH, GF], f32, kind="ExternalInput")
        w = nc.dram_tensor("w", [GF], f32, kind="ExternalInput")
        bias = nc.dram_tensor("bias", [G], f32, kind="ExternalInput")
        out = nc.dram_tensor("out", [BSH, G], f32, kind="ExternalOutput")
        _emit_v2(nc, tile, mybir, f32, x, w, bias, out)
    if not nc.is_finalized():
        nc.finalize()
    return nc


def _emit_v3(nc, tile, mybir, f32, x_t, w, bias, ones_in, out_t):
    fw = mybir.dt.float32r if V3_BCAST == "f32r" else f32
    subs = _subs()
    gset = _gsub_set()

    with tile.TileContext(nc) as tc:
        with (
            tc.tile_pool(name="const", bufs=1) as const_pool,
            tc.tile_pool(name="wrow", bufs=3) as wrow_pool,
            tc.tile_pool(name="brow", bufs=3) as brow_pool,
            tc.tile_pool(name="psum", bufs=2, space="PSUM") as psum_pool,
            tc.tile_pool(name="wg", bufs=2) as wg_pool,
            tc.tile_pool(name="x", bufs=4) as x_pool,
            tc.tile_pool(name="xwd", bufs=2) as xwd_pool,
            tc.tile_pool(name="xwg", bufs=2) as xwg_pool,
            tc.tile_pool(name="o", bufs=3) as o_pool,
        ):
            ones = const_pool.tile([1, PT], fw, tag="ones")
            nc.sync.dma_start(out=ones[:1, :], in_=ones_in[None, :])

            for si, (g0, gw) in enumerate(subs):
                cw = F * gw
                is_g = si in gset

                # x slab: [128, NT*cw], t-major within the sub-chunk
                xt = x_pool.tile([PT, NT * cw], f32, tag="x")
                nc.sync.dma_start(
                    out=xt[:, :], in_=x_t[:, NT * F * g0 : NT * F * g0 + NT * cw]
                )

                # broadcast rows + PE ones-matmuls into PSUM
                wr = wrow_pool.tile([1, F * SUB], fw, tag="wr")
                nc.sync.dma_start(
                    out=wr[:1, :], in_=w[None, F * SUB * si : F * SUB * (si + 1)]
                )
                br = brow_pool.tile([1, SUB], fw, tag="br")
                nc.sync.dma_start(
                    out=br[:1, :], in_=bias[None, SUB * si : SUB * (si + 1)]
                )
                ps = psum_pool.tile([PT, 4 * SUB], f32, tag="ps")
                for j in range(0, F * SUB, SUB):
                    nc.tensor.matmul(
                        ps[:, j : j + SUB], ones[:1, :], wr[:1, j : j + SUB]
                    )
                nc.tensor.matmul(ps[:, 3 * SUB : 4 * SUB], ones[:1, :], br[:1, :])

                if is_g:
                    # ScalarE copies w+bias PSUM->SBUF for GpSimd
                    wg = wg_pool.tile([PT, 4 * SUB], f32, tag="wg")
                    if cw == 3 * SUB:
                        nc.scalar.copy(wg[:, : 3 * SUB + gw], ps[:, : 3 * SUB + gw])
                    else:
                        nc.scalar.copy(wg[:, :cw], ps[:, :cw])
                        nc.scalar.copy(
                            wg[:, 3 * SUB : 3 * SUB + gw],
                            ps[:, 3 * SUB : 3 * SUB + gw],
                        )

                ot = o_pool.tile([PT, NT * gw], f32, tag="o")
                if V3_MERGE:
                    # one instruction per op covering all 4 batch tiles:
                    # in-place mul on the x slab, w/bias repeated across the
                    # tile dim with stride-0 broadcast APs
                    eng = nc.gpsimd if is_g else nc.vector
                    wsrc = wg if is_g else ps
                    wb = wsrc[:, None, :cw].to_broadcast((PT, NT, cw))
                    bb = wsrc[:, None, 3 * SUB : 3 * SUB + gw].to_broadcast(
                        (PT, NT, gw)
                    )
                    x3 = xt[:, :].rearrange("p (t c) -> p t c", c=cw)
                    o3 = ot[:, :].rearrange("p (t g) -> p t g", g=gw)
                    eng.tensor_mul(x3, x3, wb)
                    z = xt[:, :].rearrange("p (t g f) -> p t g f", f=F, g=gw)
                    if is_g:
                        eng.tensor_add(o3, z[:, :, :, 0], z[:, :, :, 1])
                        eng.tensor_add(o3, o3, z[:, :, :, 2])
                    else:
                        nc.vector.reduce_sum(o3, z, axis=mybir.AxisListType.X)
                    eng.tensor_add(o3, o3, bb)
                else:
                    for t in range(NT):
                        xs = xt[:, t * cw : (t + 1) * cw]
                        osl = ot[:, t * gw : (t + 1) * gw]
                        if is_g:
                            xw = xwg_pool.tile([PT, cw], f32, tag="xwg")
                            nc.gpsimd.tensor_mul(xw[:, :], xs, wg[:, :cw])
                            z = xw[:, :].rearrange("p (g f) -> p g f", f=F)
                            nc.gpsimd.tensor_add(osl, z[:, :, 0], z[:, :, 1])
                            nc.gpsimd.tensor_add(osl, osl, z[:, :, 2])
                            nc.gpsimd.tensor_add(
                                osl, osl, wg[:, 3 * SUB : 3 * SUB + gw]
                            )
                        else:
                            xw = xwd_pool.tile([PT, cw], f32, tag="xwd")
                            nc.vector.tensor_mul(xw[:, :], xs, ps[:, :cw])
                            y = xw[:, :].rearrange("p (g f) -> p g f", f=F)
                            nc.vector.reduce_sum(osl, y, axis=mybir.AxisListType.X)
                            nc.vector.tensor_add(
                                osl, osl, ps[:, 3 * SUB : 3 * SUB + gw]
                            )
                nc.scalar.dma_start(
                    out=out_t[:, NT * g0 : NT * g0 + NT * gw], in_=ot[:, :]
                )


def _emit_v2(nc, tile, mybir, f32, x, w, bias, out):
    """Previous-generation kernel (fallback). Gene-split DVE/GpSimd with
    fp32 ones-matmul broadcast via PSUM + ScalarE copies."""
    V2_GC = 2048
    V2_SPLIT = 0.68

    def chunks():
        out_ = []
        c0 = 0
        while c0 < G:
            gc = min(V2_GC, G - c0)
            out_.append((c0, gc))
            c0 += gc
        return out_

    with tile.TileContext(nc) as tc:
        with (
            tc.tile_pool(name="const", bufs=1) as const_pool,
            tc.tile_pool(name="wrow", bufs=2) as row_pool,
            tc.tile_pool(name="psum", bufs=6, space="PSUM") as psum_pool,
            tc.tile_pool(name="wb", bufs=2) as wb_pool,
            tc.tile_pool(name="bb", bufs=2) as bb_pool,
            tc.tile_pool(name="xa", bufs=3) as xa_pool,
            tc.tile_pool(name="xb", bufs=3) as xb_pool,
            tc.tile_pool(name="oa", bufs=4) as oa_pool,
            tc.tile_pool(name="ob", bufs=4) as ob_pool,
        ):
            ones = const_pool.tile([1, PT], f32, tag="ones")
            nc.vector.memset(ones[:, :], 1.0)
            ROW = 1024

            def bcast(dst, src_dram, off, n_total):
                for o in range(0, n_total, ROW):
                    n = min(ROW, n_total - o)
                    row = row_pool.tile([1, ROW], f32, tag="wrow")
                    nc.sync.dma_start(
                        out=row[:1, :n], in_=src_dram[None, off + o : off + o + n]
                    )
                    for o2 in range(0, n, 512):
                        n2 = min(512, n - o2)
                        ps = psum_pool.tile([PT, 512], f32, tag="ps")
                        nc.tensor.matmul(
                            ps[:, :n2], ones[:1, :], row[:1, o2 : o2 + n2]
                        )
                        nc.scalar.copy(dst[:, o + o2 : o + o2 + n2], ps[:, :n2])

            def bcast_chunk(c0, gc):
                wbt = wb_pool.tile([PT, F * gc], f32, tag="wb")
                bcast(wbt, w, F * c0, F * gc)
                bbt = bb_pool.tile([PT, gc], f32, tag="bb")
                bcast(bbt, bias, c0, gc)
                return wbt, bbt

            chs = chunks()
            cur = bcast_chunk(*chs[0])
            for ci, (c0, gc) in enumerate(chs):
                wbt, bbt = cur
                s = int(round(gc * V2_SPLIT))
                nb = gc - s
                for t in range(NT):
                    rows = slice(t * PT, (t + 1) * PT)
                    xa_t = xa_pool.tile([PT, F * s], f32, tag="xa")
                    nc.sync.dma_start(
                        out=xa_t[:, :], in_=x[rows, F * c0 : F * (c0 + s)]
                    )
                    oa_t = oa_pool.tile([PT, s], f32, tag="oa")
                    nc.vector.tensor_mul(xa_t[:, :], xa_t[:, :], wbt[:, : F * s])
                    y3 = xa_t[:, :].rearrange("p (g f) -> p g f", f=F)
                    nc.vector.reduce_sum(oa_t[:, :], y3, axis=mybir.AxisListType.X)
                    nc.vector.tensor_add(oa_t[:, :], oa_t[:, :], bbt[:, :s])
                    nc.scalar.dma_start(out=out[rows, c0 : c0 + s], in_=oa_t[:, :])

                    xb_t = xb_pool.tile([PT, F * nb], f32, tag="xb")
                    nc.sync.dma_start(
                        out=xb_t[:, :], in_=x[rows, F * (c0 + s) : F * (c0 + gc)]
                    )
                    ob_t = ob_pool.tile([PT, nb], f32, tag="ob")
                    nc.gpsimd.tensor_mul(
                        xb_t[:, :], xb_t[:, :], wbt[:, F * s : F * gc]
                    )
                    z3 = xb_t[:, :].rearrange("p (g f) -> p g f", f=F)
                    nc.gpsimd.tensor_add(ob_t[:, :], z3[:, :, 0], z3[:, :, 1])
                    nc.gpsimd.tensor_add(ob_t[:, :], ob_t[:, :], z3[:, :, 2])
                    nc.gpsimd.tensor_add(ob_t[:, :], ob_t[:, :], bbt[:, s:gc])
                    nc.scalar.dma_start(
                        out=out[rows, c0 + s : c0 + gc], in_=ob_t[:, :]
                    )
                    if t == 0 and ci + 1 < len(chs):
                        cur = bcast_chunk(*chs[ci + 1])


def _get_nc():
    global _cached_nc
    if _cached_nc is None:
        _cached_nc = _build_nc()
    return _cached_nc


def _pack_x(xc):
    """[512, GF] -> [128, NT*GF]: per sub-chunk, t-major slabs."""
    x3 = xc.reshape(NT, PT, GF)
    slabs = []
    for g0, gw in _subs():
        blk = x3[:, :, F * g0 : F * (g0 + gw)]  # [NT, PT, cw]
        slabs.append(blk.transpose(1, 0, 2).reshape(PT, NT * F * gw))
    return np.ascontiguousarray(np.concatenate(slabs, axis=1))


def _unpack_out(ot):
    """[128, NT*G] -> [512, G]"""
    cols = []
    for g0, gw in _subs():
        blk = ot[:, NT * g0 : NT * (g0 + gw)].reshape(PT, NT, gw)
        cols.append(blk.transpose(1, 0, 2).reshape(BSH, gw))
    return np.concatenate(cols, axis=1)


def run(x, weights, bias, trace=False, tmpdir=None):
    from concourse.bass_utils import run_bass_kernel_spmd

    x = np.ascontiguousarray(np.asarray(x, dtype=np.float32))
    weights = np.ascontiguousarray(np.asarray(weights, dtype=np.float32))
    bias_np = np.ascontiguousarray(np.asarray(bias, dtype=np.float32))

    nc = _get_nc()
    if VARIANT == "v3":
        ones128 = np.ones(PT, dtype=np.float32)
        w_pad = np.zeros(F * GPAD, dtype=np.float32)
        w_pad[:GF] = weights
        b_pad = np.zeros(GPAD, dtype=np.float32)
        b_pad[:G] = bias_np
        in_maps = [
            {
                "x_t": _pack_x(x[c * BSH : (c + 1) * BSH]),
                "w": w_pad,
                "bias": b_pad,
                "ones_in": ones128,
            }
            for c in range(NCORES)
        ]
    else:
        in_maps = [
            {
                "x": np.ascontiguousarray(x[c * BSH : (c + 1) * BSH]),
                "w": weights,
                "bias": bias_np,
            }
            for c in range(NCORES)
        ]
    try:
        res = run_bass_kernel_spmd(
            nc, in_maps, list(range(NCORES)), trace=trace, tmpdir=tmpdir
        )
    except Exception:
        # transient NRT device errors usually clear on retry
        res = run_bass_kernel_spmd(
            nc, in_maps, list(range(NCORES)), trace=trace, tmpdir=tmpdir
        )
    if VARIANT == "v3":
        outs = [_unpack_out(res.results[c]["out_t"]) for c in range(NCORES)]
    else:
        outs = [res.results[c]["out"] for c in range(NCORES)]
    full = np.concatenate(outs, axis=0)
    return full, res


def kernel(x, weights, bias):
    full, _ = run(x, weights, bias, trace=False)
    return full
